# revision 6
# baseline (speedup 1.0000x reference)
"""Trainium2 Bass kernel for nn_DecodePredictions (YOLO-style decode, B=16).

Pure data-parallel over batch (2 images per core x 8 cores).

The [B, N*C, 6] output is hugely redundant on the device side: per anchor,
the 4 box coords repeat across all 80 classes and lane 4 is the constant
class id. The device therefore emits only the per-anchor uniques --
4 box coords (bf16) and 80 class scores (bf16) -- ~1.5 MB/core instead of
the 16 MB/core the full layout costs; the host broadcasts them into the
full [B, N*C, 6] fp32 array while unsharding.

Scores: sigma on ACT (the only engine with activation LUTs), then one
tensor_tensor multiply per class chunk against broadcast sigma(obj) --
all-bf16 step-1 APs keep the DVE in its 2x packed mode (the fused
scalar_tensor_tensor alternative only has a 1x uop and measures 2x
slower). Box wh avoids the Exp table entirely -- exp(w) =
sigma(w)/(1-sigma(w)) via the DVE's hardware-divide reciprocal on the
tiny [P, 264] plane -- so the whole kernel uses ONE ACT table set and
pays a single ACT_TABLE_LOAD, which hides under the NEFF preamble.

Layouts are class-major [P, 81, KPP] (anchor innermost) so every ACT/DVE
op is step-1 contiguous and every DMA moves 128 contiguous per-partition
segments of 2-5 KB. Input DMAs are split across both HWDGE rings (sync +
scalar sequencers issue descriptors concurrently, ~600ns each); score
chunks stream out as their multiplies finish, boxes slot into the out
ring mid-stream, and the last class chunk is small to shorten the
drain tail.
"""

import ml_dtypes
import numpy as np

N_CORES = 8
B = 16
B_PER_CORE = B // N_CORES  # 2
C = 80
F = 85
N_REAL = 8400              # 80*80 + 40*40 + 20*20
N_PAD = 8448               # = 66 * 128
P = 128
KPP = B_PER_CORE * N_PAD // P  # 132 anchors per partition
R = C + 1                  # obj row + 80 class rows
# sigma-row chunks [r0, r1); chunk 0 carries the obj row. Score rows are
# the same ranges shifted down by one, so multiply chunk c depends only on
# sigma chunk c (plus sigma(obj) from chunk 0). First and last chunks are
# small: chunk 0's DMA latency heads the ACT critical chain, and the last
# chunk's sigma->mul->DMA chain is the drain tail.
SCH = [(0, 8), (8, 26), (26, 44), (44, 62), (62, 76), (76, 81)]
# input DMA row chunks (sigma chunks wait on whichever transfers cover them)
DCH = [(0, 8), (8, 30), (30, 55), (55, 81)]

_CACHE: dict = {}


def _build_nc():
    import concourse.bacc as bacc
    import concourse.tile as tile
    from concourse import mybir
    from contextlib import ExitStack

    nc = bacc.Bacc("TRN2", target_bir_lowering=False, debug=False)
    predsT = nc.dram_tensor("predsT", [P, R, KPP], mybir.dt.float8e4, kind="ExternalInput")
    pa = nc.dram_tensor("pa", [P, 4, KPP], mybir.dt.float32, kind="ExternalInput")
    aux = nc.dram_tensor("aux", [P, 4, KPP], mybir.dt.bfloat16, kind="ExternalInput")
    scores = nc.dram_tensor("scores", [P, C, KPP], mybir.dt.bfloat16, kind="ExternalOutput")
    boxes = nc.dram_tensor("boxes", [P, 4, KPP], mybir.dt.bfloat16, kind="ExternalOutput")

    fp32 = mybir.dt.float32
    bf16 = mybir.dt.bfloat16
    AF = mybir.ActivationFunctionType
    OP = mybir.AluOpType

    with tile.TileContext(nc) as tc, ExitStack() as ctx:
        pool = ctx.enter_context(tc.tile_pool(name="m", bufs=1))

        pt = pool.tile([P, R, KPP], mybir.dt.float8e4, tag="pt")
        pa_t = pool.tile([P, 4, KPP], fp32, tag="pa")
        aux_t = pool.tile([P, 4, KPP], bf16, tag="aux")

        # Input DMAs split across both HWDGE rings so descriptor issue
        # (~600ns each, serialized per sequencer) runs in parallel, and the
        # preds stream (the ACT critical chain) is not queued behind the
        # box inputs: sync ring takes all preds row chunks in order, the
        # scalar ring takes pa + aux (only needed by the box path).
        for r0, r1 in DCH:
            nc.sync.dma_start(out=pt[:, r0:r1, :], in_=predsT[:, r0:r1, :])
        nc.scalar.dma_start(out=pa_t[:], in_=pa[:])
        nc.scalar.dma_start(out=aux_t[:], in_=aux[:])

        sg = pool.tile([P, R, KPP], bf16, tag="sg")
        sp = pool.tile([P, 2, KPP], fp32, tag="sp")

        # ACT ladder: sigma chunks 0-1, the tiny box sigma, then the rest.
        for r0, r1 in SCH[:2]:
            nc.scalar.activation(sg[:, r0:r1, :], pt[:, r0:r1, :], AF.Sigmoid)
        nc.scalar.activation(sp[:], pa_t[:, 2:4, :], AF.Sigmoid)
        for r0, r1 in SCH[2:]:
            nc.scalar.activation(sg[:, r0:r1, :], pt[:, r0:r1, :], AF.Sigmoid)

        sc_t = pool.tile([P, C, KPP], bf16, tag="sc")
        om_t = pool.tile([P, 2, KPP], fp32, tag="om")
        rc_t = pool.tile([P, 2, KPP], fp32, tag="rc")
        wh_t = pool.tile([P, 2, KPP], fp32, tag="wh")
        xy_s = pool.tile([P, 2, KPP], fp32, tag="xys")
        xy1 = pool.tile([P, 2, KPP], fp32, tag="xy1")
        wh_s = pool.tile([P, 2, KPP], fp32, tag="whs")
        box_t = pool.tile([P, 4, KPP], bf16, tag="box")

        def score_chunk(c):
            r0, r1 = SCH[c]
            s0, t0 = (0, 1) if c == 0 else (r0 - 1, r0)
            s1 = r1 - 1
            nc.vector.tensor_mul(
                sc_t[:, s0:s1, :],
                sg[:, t0:r1, :],
                sg[:, 0, :].unsqueeze(1).broadcast_to([P, s1 - s0, KPP]),
            )
            # Alternate output rings so score transfers drain in parallel.
            eng = nc.scalar if c % 2 == 0 else nc.sync
            eng.dma_start(out=scores[:, s0:s1, :], in_=sc_t[:, s0:s1, :])

        score_chunk(0)
        score_chunk(1)

        # Box decode: x1 = px*s + bx, x2 = x1 + exp(pw)*s (y alike; the
        # /image_shape then *W,H of the reference cancels, H == W == 640).
        # exp(w) = sigma(w) / (1 - sigma(w)), reciprocal on DVE.
        nc.vector.tensor_scalar(om_t[:], sp[:], 1.0, -1.0, OP.subtract, OP.mult)
        nc.vector.reciprocal(rc_t[:], om_t[:])
        nc.vector.tensor_mul(wh_t[:], sp[:], rc_t[:])
        nc.vector.tensor_mul(xy_s[:], pa_t[:, 0:2, :], aux_t[:, 0:2, :])
        nc.vector.tensor_add(xy1[:], xy_s[:], aux_t[:, 2:4, :])
        nc.vector.tensor_copy(box_t[:, 0:2, :], xy1[:])
        nc.vector.tensor_mul(wh_s[:], wh_t[:], aux_t[:, 0:2, :])
        nc.vector.tensor_add(box_t[:, 2:4, :], xy1[:], wh_s[:])
        nc.scalar.dma_start(out=boxes[:], in_=box_t[:])

        for c in range(2, len(SCH)):
            score_chunk(c)

    nc.compile()
    return nc


def _host_consts():
    # Per-anchor stride s and grid offsets bx = gx*s, by = gy*s, padded to
    # N_PAD, replicated for the 2 images per core, as [P, 4, KPP] planes
    # (s, s, bx, by). All values are exact in bf16.
    s = np.ones(N_PAD, np.float32)
    bx = np.zeros(N_PAD, np.float32)
    by = np.zeros(N_PAD, np.float32)
    off = 0
    for g, st in ((80, 8.0), (40, 16.0), (20, 32.0)):
        n = g * g
        i = np.arange(n)
        s[off : off + n] = st
        bx[off : off + n] = (i % g) * st
        by[off : off + n] = (i // g) * st
        off += n
    pl = np.stack([s, s, bx, by], 0)                     # [4, N_PAD]
    pl = np.concatenate([pl] * B_PER_CORE, 1)            # [4, 2*N_PAD]
    aux = pl.reshape(4, P, KPP).transpose(1, 0, 2)       # [P, 4, KPP]
    return np.ascontiguousarray(aux.astype(ml_dtypes.bfloat16))


def _host_in_maps(pred0, pred1, pred2):
    aux = _CACHE["consts"]
    pred0 = np.asarray(pred0, np.float32).reshape(B, -1, F)
    pred1 = np.asarray(pred1, np.float32).reshape(B, -1, F)
    pred2 = np.asarray(pred2, np.float32).reshape(B, -1, F)
    in_maps = []
    for core in range(N_CORES):
        flat = np.zeros((B_PER_CORE * N_PAD, F), np.float32)
        for j in range(B_PER_CORE):
            b = core * B_PER_CORE + j
            flat[j * N_PAD : j * N_PAD + N_REAL] = np.concatenate(
                [pred0[b], pred1[b], pred2[b]], axis=0
            )
        a = flat.reshape(P, KPP, F)                      # [p, k, field]
        predsT = np.empty((P, R, KPP), np.float32)
        predsT[:, 0, :] = a[:, :, 4]
        predsT[:, 1:, :] = a[:, :, 5:].transpose(0, 2, 1)
        in_maps.append(
            {
                "predsT": predsT.astype(ml_dtypes.float8_e4m3fn),
                "pa": np.ascontiguousarray(a[:, :, 0:4].transpose(0, 2, 1)),
                "aux": aux,
            }
        )
    return in_maps


def kernel(images, pred0, pred1, pred2):
    from concourse.bass_utils import run_bass_kernel_spmd

    if "nc" not in _CACHE:
        _CACHE["consts"] = _host_consts()
        _CACHE["nc"] = _build_nc()
    nc = _CACHE["nc"]

    in_maps = _host_in_maps(pred0, pred1, pred2)
    res = run_bass_kernel_spmd(nc, in_maps, list(range(N_CORES)))

    final = np.empty((B, N_REAL * C, 6), np.float32)
    v = final.reshape(B, N_REAL, C, 6)
    v[..., 4] = np.arange(C, dtype=np.float32)[None, None, :]
    for core, r in enumerate(res.results):
        # [P, C, KPP] -> per-image [N_REAL, C]; [P, 4, KPP] -> [N_REAL, 4]
        sc = (
            r["scores"].astype(np.float32)
            .reshape(B_PER_CORE, P // B_PER_CORE, C, KPP)
            .transpose(0, 1, 3, 2)
            .reshape(B_PER_CORE, N_PAD, C)
        )
        bx = (
            r["boxes"].astype(np.float32)
            .reshape(B_PER_CORE, P // B_PER_CORE, 4, KPP)
            .transpose(0, 1, 3, 2)
            .reshape(B_PER_CORE, N_PAD, 4)
        )
        for j in range(B_PER_CORE):
            b = core * B_PER_CORE + j
            v[b, :, :, 0:4] = bx[j, :N_REAL, None, :]
            v[b, :, :, 5] = sc[j, :N_REAL, :]
    return final


# revision 9
# speedup vs baseline: 1.0681x; 1.0681x over previous
"""Trainium2 Bass kernel for nn_DecodePredictions (YOLO-style decode, B=16).

Pure data-parallel over batch (2 images per core x 8 cores).

The [B, N*C, 6] output is hugely redundant on the device side: per anchor,
the 4 box coords repeat across all 80 classes and lane 4 is the constant
class id. The device therefore emits only the per-anchor uniques --
4 box coords (bf16) and 80 class scores (bf16) -- ~1.5 MB/core instead of
the 16 MB/core the full layout costs; the host broadcasts them into the
full [B, N*C, 6] fp32 array while unsharding.

Scores: sigma on ACT (the only engine with activation LUTs), then one
tensor_tensor multiply per class chunk against broadcast sigma(obj) --
all-bf16 step-1 APs keep the DVE in its 2x packed mode (the fused
scalar_tensor_tensor alternative only has a 1x uop and measures 2x
slower). Box wh avoids the Exp table entirely -- exp(w) =
sigma(w)/(1-sigma(w)) via the DVE's hardware-divide reciprocal on the
tiny [P, 264] plane -- so the whole kernel uses ONE ACT table set and
pays a single ACT_TABLE_LOAD, which hides under the NEFF preamble.

Layouts are class-major [P, 81, KPP] (anchor innermost) so every ACT/DVE
op is step-1 contiguous and every DMA moves 128 contiguous per-partition
segments of 2-5 KB. Input DMAs are split across both HWDGE rings (sync +
scalar sequencers issue descriptors concurrently, ~600ns each); score
chunks stream out as their multiplies finish, boxes slot into the out
ring mid-stream, and the last class chunk is small to shorten the
drain tail.
"""

import ml_dtypes
import numpy as np

N_CORES = 8
B = 16
B_PER_CORE = B // N_CORES  # 2
C = 80
F = 85
N_REAL = 8400              # 80*80 + 40*40 + 20*20
N_PAD = 8448               # = 66 * 128
P = 128
KPP = B_PER_CORE * N_PAD // P  # 132 anchors per partition
R = C + 1                  # obj row + 80 class rows
# sigma-row chunks [r0, r1); chunk 0 carries the obj row. Score rows are
# the same ranges shifted down by one, so multiply chunk c depends only on
# sigma chunk c (plus sigma(obj) from chunk 0). First and last chunks are
# small: chunk 0's DMA latency heads the ACT critical chain, and the last
# chunk's sigma->mul->DMA chain is the drain tail.
SCH = [(0, 8), (8, 22), (22, 40), (40, 58), (58, 76), (76, 81)]
# input DMA row chunks (sigma chunks wait on whichever transfers cover them)
DCH = [(0, 8), (8, 22), (22, 40), (40, 58), (58, 81)]

_CACHE: dict = {}


def _build_nc():
    import concourse.bacc as bacc
    import concourse.tile as tile
    from concourse import mybir
    from contextlib import ExitStack

    nc = bacc.Bacc("TRN2", target_bir_lowering=False, debug=False)
    predsT = nc.dram_tensor("predsT", [P, R, KPP], mybir.dt.float8e4, kind="ExternalInput")
    pa = nc.dram_tensor("pa", [P, 4, KPP], mybir.dt.float32, kind="ExternalInput")
    aux = nc.dram_tensor("aux", [P, 4, KPP], mybir.dt.bfloat16, kind="ExternalInput")
    scores = nc.dram_tensor("scores", [P, C, KPP], mybir.dt.bfloat16, kind="ExternalOutput")
    boxes = nc.dram_tensor("boxes", [P, 4, KPP], mybir.dt.bfloat16, kind="ExternalOutput")

    fp32 = mybir.dt.float32
    bf16 = mybir.dt.bfloat16
    AF = mybir.ActivationFunctionType
    OP = mybir.AluOpType

    with tile.TileContext(nc) as tc, ExitStack() as ctx:
        pool = ctx.enter_context(tc.tile_pool(name="m", bufs=1))

        pt = pool.tile([P, R, KPP], mybir.dt.float8e4, tag="pt")
        pa_t = pool.tile([P, 4, KPP], fp32, tag="pa")
        aux_t = pool.tile([P, 4, KPP], bf16, tag="aux")

        # ALL DMAs ride the sync HWDGE ring: the scalar sequencer then
        # carries a pure ACT stream (table loads + sigmoids) so descriptor
        # issue (~600ns each) never delays an activation dispatch, and the
        # two ACT_TABLE_LOADs finish (~9.7us) right as preds chunk 0 lands.
        # Input order: preds chunks sized so each sigma's data lands just
        # before the ladder reaches it; pa/aux (box path) slot mid-stream.
        for r0, r1 in DCH[:3]:
            nc.sync.dma_start(out=pt[:, r0:r1, :], in_=predsT[:, r0:r1, :])
        nc.sync.dma_start(out=pa_t[:], in_=pa[:])
        nc.sync.dma_start(out=pt[:, DCH[3][0] : DCH[3][1], :],
                          in_=predsT[:, DCH[3][0] : DCH[3][1], :])
        nc.sync.dma_start(out=aux_t[:], in_=aux[:])
        nc.sync.dma_start(out=pt[:, DCH[4][0] : DCH[4][1], :],
                          in_=predsT[:, DCH[4][0] : DCH[4][1], :])

        sg = pool.tile([P, R, KPP], bf16, tag="sg")
        sp = pool.tile([P, 2, KPP], fp32, tag="sp")

        # ACT ladder: sigma chunks 0-1, the tiny box sigma, then the rest.
        for r0, r1 in SCH[:2]:
            nc.scalar.activation(sg[:, r0:r1, :], pt[:, r0:r1, :], AF.Sigmoid)
        nc.scalar.activation(sp[:], pa_t[:, 2:4, :], AF.Sigmoid)
        for r0, r1 in SCH[2:]:
            nc.scalar.activation(sg[:, r0:r1, :], pt[:, r0:r1, :], AF.Sigmoid)

        sc_t = pool.tile([P, C, KPP], bf16, tag="sc")
        om_t = pool.tile([P, 2, KPP], fp32, tag="om")
        rc_t = pool.tile([P, 2, KPP], fp32, tag="rc")
        wh_t = pool.tile([P, 2, KPP], fp32, tag="wh")
        xy_s = pool.tile([P, 2, KPP], fp32, tag="xys")
        xy1 = pool.tile([P, 2, KPP], fp32, tag="xy1")
        wh_s = pool.tile([P, 2, KPP], fp32, tag="whs")
        box_t = pool.tile([P, 4, KPP], bf16, tag="box")

        def score_chunk(c):
            r0, r1 = SCH[c]
            s0, t0 = (0, 1) if c == 0 else (r0 - 1, r0)
            s1 = r1 - 1
            nc.vector.tensor_mul(
                sc_t[:, s0:s1, :],
                sg[:, t0:r1, :],
                sg[:, 0, :].unsqueeze(1).broadcast_to([P, s1 - s0, KPP]),
            )
            nc.sync.dma_start(out=scores[:, s0:s1, :], in_=sc_t[:, s0:s1, :])

        score_chunk(0)
        score_chunk(1)

        # Box decode: x1 = px*s + bx, x2 = x1 + exp(pw)*s (y alike; the
        # /image_shape then *W,H of the reference cancels, H == W == 640).
        # exp(w) = sigma(w) / (1 - sigma(w)), reciprocal on DVE. The xy
        # path runs on the otherwise-idle GpSimd so the DVE stays clear
        # for the score multiplies; the DVE's recip chain fills its gap
        # between score chunks 1 and 2.
        nc.vector.tensor_scalar(om_t[:], sp[:], 1.0, -1.0, OP.subtract, OP.mult)
        nc.vector.reciprocal(rc_t[:], om_t[:])
        nc.gpsimd.tensor_mul(xy_s[:], pa_t[:, 0:2, :], aux_t[:, 0:2, :])
        nc.gpsimd.tensor_add(xy1[:], xy_s[:], aux_t[:, 2:4, :])
        nc.gpsimd.tensor_copy(box_t[:, 0:2, :], xy1[:])

        score_chunk(2)
        nc.vector.tensor_mul(wh_t[:], sp[:], rc_t[:])
        nc.vector.tensor_mul(wh_s[:], wh_t[:], aux_t[:, 0:2, :])
        nc.vector.tensor_add(box_t[:, 2:4, :], xy1[:], wh_s[:])
        score_chunk(3)
        nc.sync.dma_start(out=boxes[:], in_=box_t[:])
        score_chunk(4)
        score_chunk(5)

    nc.compile()
    return nc


def _host_consts():
    # Per-anchor stride s and grid offsets bx = gx*s, by = gy*s, padded to
    # N_PAD, replicated for the 2 images per core, as [P, 4, KPP] planes
    # (s, s, bx, by). All values are exact in bf16.
    s = np.ones(N_PAD, np.float32)
    bx = np.zeros(N_PAD, np.float32)
    by = np.zeros(N_PAD, np.float32)
    off = 0
    for g, st in ((80, 8.0), (40, 16.0), (20, 32.0)):
        n = g * g
        i = np.arange(n)
        s[off : off + n] = st
        bx[off : off + n] = (i % g) * st
        by[off : off + n] = (i // g) * st
        off += n
    pl = np.stack([s, s, bx, by], 0)                     # [4, N_PAD]
    pl = np.concatenate([pl] * B_PER_CORE, 1)            # [4, 2*N_PAD]
    aux = pl.reshape(4, P, KPP).transpose(1, 0, 2)       # [P, 4, KPP]
    return np.ascontiguousarray(aux.astype(ml_dtypes.bfloat16))


def _host_in_maps(pred0, pred1, pred2):
    aux = _CACHE["consts"]
    pred0 = np.asarray(pred0, np.float32).reshape(B, -1, F)
    pred1 = np.asarray(pred1, np.float32).reshape(B, -1, F)
    pred2 = np.asarray(pred2, np.float32).reshape(B, -1, F)
    in_maps = []
    for core in range(N_CORES):
        flat = np.zeros((B_PER_CORE * N_PAD, F), np.float32)
        for j in range(B_PER_CORE):
            b = core * B_PER_CORE + j
            flat[j * N_PAD : j * N_PAD + N_REAL] = np.concatenate(
                [pred0[b], pred1[b], pred2[b]], axis=0
            )
        a = flat.reshape(P, KPP, F)                      # [p, k, field]
        predsT = np.empty((P, R, KPP), np.float32)
        predsT[:, 0, :] = a[:, :, 4]
        predsT[:, 1:, :] = a[:, :, 5:].transpose(0, 2, 1)
        in_maps.append(
            {
                "predsT": predsT.astype(ml_dtypes.float8_e4m3fn),
                "pa": np.ascontiguousarray(a[:, :, 0:4].transpose(0, 2, 1)),
                "aux": aux,
            }
        )
    return in_maps


def kernel(images, pred0, pred1, pred2):
    from concourse.bass_utils import run_bass_kernel_spmd

    if "nc" not in _CACHE:
        _CACHE["consts"] = _host_consts()
        _CACHE["nc"] = _build_nc()
    nc = _CACHE["nc"]

    in_maps = _host_in_maps(pred0, pred1, pred2)
    res = run_bass_kernel_spmd(nc, in_maps, list(range(N_CORES)))

    final = np.empty((B, N_REAL * C, 6), np.float32)
    v = final.reshape(B, N_REAL, C, 6)
    v[..., 4] = np.arange(C, dtype=np.float32)[None, None, :]
    for core, r in enumerate(res.results):
        # [P, C, KPP] -> per-image [N_REAL, C]; [P, 4, KPP] -> [N_REAL, 4]
        sc = (
            r["scores"].astype(np.float32)
            .reshape(B_PER_CORE, P // B_PER_CORE, C, KPP)
            .transpose(0, 1, 3, 2)
            .reshape(B_PER_CORE, N_PAD, C)
        )
        bx = (
            r["boxes"].astype(np.float32)
            .reshape(B_PER_CORE, P // B_PER_CORE, 4, KPP)
            .transpose(0, 1, 3, 2)
            .reshape(B_PER_CORE, N_PAD, 4)
        )
        for j in range(B_PER_CORE):
            b = core * B_PER_CORE + j
            v[b, :, :, 0:4] = bx[j, :N_REAL, None, :]
            v[b, :, :, 5] = sc[j, :N_REAL, :]
    return final
